# revision 4
# baseline (speedup 1.0000x reference)
"""Trainium2 Bass kernel for nn_DifferentiableProjector (volume rendering).

Math (per ray i, samples s=0..S-1, channels c):
    T_excl[s] = exp(-DT * sum_{s'<s} rho[s'])
    T_incl[s] = exp(-DT * sum_{s'<=s} rho[s'])
    w[s]      = T_excl[s] - T_incl[s]        (= T_excl * alpha)
    out[i,c]  = sum_s w[s] * f[i,s,c]

Sharding: data-parallel over rays, 65536 rays -> 8 cores x 8192 rays.

v3 design (all compute in "transposed space", s on partitions):
  - w for ALL tiles is precomputed up front from rho alone (cumsum via
    triangular matmuls -> fp32 exps -> DVE sub).  This decouples the
    rho->w pipeline from the f pipeline so no cross-tile FIFO convoys
    form (v2 lost ~25 us to one).
  - f tiles 0-1 are fp16 via HWDGE (starts ~2.6us, before the SWDGE
    preamble); tiles 2+ are float8_e3m4 in DRAM upcast to fp16 DURING
    the HBM->SBUF DMA (SWDGE cast path).  SBUF-write side (~400 GB/s)
    is the binding resource; fp8 halves HBM reads and eases it.
  - the big multiply runs on VectorE at 2x fp16
  - segment-reduce over s: accumulating one-hot matmuls on TensorE,
    interleaved per quarter so the reduce trails each quarter-multiply
  - ScalarE evacuates PSUM -> SBUF fp16; out DMAs go on the idle sync
    queue; rho loads in 4 chunks so w-precompute starts early
  - two idle-time probe ops measure GPSIMD fp16 and DVE fp8-operand
    multiply rates for future tuning (no effect on results)
"""

import numpy as np
import ml_dtypes

import concourse.bass as bass
import concourse.tile as tile
from concourse.bacc import Bacc
from concourse import mybir
from concourse.bass_utils import run_bass_kernel_spmd

H, W, S, C = 256, 256, 128, 16
N = H * W
NCORES = 8
NS = N // NCORES          # rays per core
P = 128                   # partitions (= S)
T = 512                   # rays per tile
DT = (6.0 - 2.0) / S
NH = 2                    # head tiles kept fp16 (HWDGE path)

_cached = {}

# test-harness hooks (ignored by grading path)
TRACE = False
LAST_RESULTS = None

F8 = mybir.dt.float8e3
F16 = mybir.dt.float16
F32 = mybir.dt.float32


def _build_nc(ns: int = NS) -> bass.Bass:
    ntiles = ns // T
    nc = Bacc()
    # host supplies tensors pre-transposed AND pre-tiled:
    #   rho  [ntiles*S, T]        fp16
    #   f16h [NH*S, C*T]          fp16      (tiles 0..NH-1)
    #   f8   [(ntiles-NH)*S, C*T] float8e3  (tiles NH..)
    rho_d = nc.dram_tensor("rho", [ntiles * S, T], F16, kind="ExternalInput")
    f16h_d = nc.dram_tensor("f16h", [NH * S, C * T], F16, kind="ExternalInput")
    f8_d = nc.dram_tensor(
        "f8", [(ntiles - NH) * S, C * T], F8, kind="ExternalInput"
    )
    cst_d = nc.dram_tensor("consts", [P, 2 * P + C * C], F16, kind="ExternalInput")
    out_d = nc.dram_tensor("out", [C, ns], F16, kind="ExternalOutput")

    with tile.TileContext(nc) as tc:
        with (
            tc.tile_pool(name="cpool", bufs=1) as cpool,
            tc.tile_pool(name="fpool", bufs=5) as fpool,
            tc.tile_pool(name="tpool", bufs=3) as tpool,
            tc.tile_pool(name="spool", bufs=4) as spool,
            tc.tile_pool(name="opool", bufs=3) as opool,
            tc.tile_pool(name="prb", bufs=1) as prb,
            tc.tile_pool(name="psc", bufs=2, space="PSUM") as psc,
            tc.tile_pool(name="pso", bufs=2, space="PSUM") as pso,
        ):
            consts = cpool.tile([P, 2 * P + C * C], F16)
            nc.scalar.dma_start(out=consts, in_=cst_d[:, :])
            u_excl = consts[:, 0:P]
            u_incl = consts[:, P : 2 * P]
            e_base = 2 * P

            # rho in 4 chunks so the w-precompute starts after ~0.5 MB
            rho_all = cpool.tile([P, ntiles, T], F16)
            RCH = 4
            for rt in range(0, ntiles, RCH):
                nc.sync.dma_start(
                    out=rho_all[:, rt : rt + RCH, :],
                    in_=rho_d[rt * S : (rt + RCH) * S, :].rearrange(
                        "(t s) i -> s t i", s=S
                    ),
                )

            # f tile DMAs, issued up front (buffer ring gates the tail)
            fts = []
            for t in range(ntiles):
                fT = fpool.tile([P, C, T], F16, tag="fT")
                if t < NH:
                    nc.scalar.dma_start(
                        out=fT,
                        in_=f16h_d[t * S : (t + 1) * S, :].rearrange(
                            "s (c i) -> s c i", c=C
                        ),
                    )
                else:
                    nc.gpsimd.dma_start(
                        out=fT,
                        in_=f8_d[(t - NH) * S : (t - NH + 1) * S, :].rearrange(
                            "s (c i) -> s c i", c=C
                        ),
                    )
                fts.append(fT)

            # probe input: one raw-fp8 slab via HWDGE (same-dtype, no cast)
            p8 = prb.tile([P, T], F8)
            nc.sync.dma_start(out=p8, in_=f8_d[0:P, 0:T])

            # ---- w precompute for all tiles (depends only on rho) ----
            w_all = cpool.tile([P, ntiles, T], F16)
            for t in range(ntiles):
                pexc = psc.tile([P, T], F32, tag="pexc")
                pinc = psc.tile([P, T], F32, tag="pinc")
                rhoT = rho_all[:, t, :]
                nc.tensor.matmul(pexc, u_excl, rhoT, start=True, stop=True)
                nc.tensor.matmul(pinc, u_incl, rhoT, start=True, stop=True)
                e1 = spool.tile([P, T], F32, tag="e1")
                e2 = spool.tile([P, T], F32, tag="e2")
                nc.scalar.activation(
                    e1, pexc, mybir.ActivationFunctionType.Exp, scale=-DT
                )
                nc.scalar.activation(
                    e2, pinc, mybir.ActivationFunctionType.Exp, scale=-DT
                )
                nc.vector.tensor_sub(w_all[:, t, :], e1, e2)

            # ---- f pipeline: multiply + per-quarter reduce per pair ----
            NQ = 4
            CQ = C // NQ
            for t0 in range(0, ntiles, 2):
                pair = (t0, t0 + 1)
                tmps = []
                for t in pair:
                    tmp = tpool.tile([P, C, T], F16, tag="tmp")
                    for q in range(NQ):
                        c0 = q * CQ
                        nc.vector.tensor_mul(
                            tmp[:, c0 : c0 + CQ, :],
                            fts[t][:, c0 : c0 + CQ, :],
                            w_all[:, t, None, :].broadcast_to((P, CQ, T)),
                        )
                    tmps.append(tmp)
                psums = [
                    pso.tile([C, T], F32, tag=f"po{i}", name=f"po{i}_{t0}")
                    for i in range(2)
                ]
                for q in range(NQ):
                    for c in range(q * CQ, (q + 1) * CQ):
                        lhs = consts[:, e_base + c * C : e_base + (c + 1) * C]
                        for tmp_t, po in zip(tmps, psums):
                            nc.tensor.matmul(
                                po,
                                lhs,
                                tmp_t[:, c, :],
                                start=(c == 0),
                                stop=(c == C - 1),
                            )
                out_pair = opool.tile(
                    [C, 2 * T], F16, tag="out_pair", name=f"out_pair_{t0}"
                )
                for k, po in enumerate(psums):
                    nc.scalar.activation(
                        out_pair[:, k * T : (k + 1) * T],
                        po,
                        mybir.ActivationFunctionType.Copy,
                    )
                nc.sync.dma_start(
                    out=out_d[:, t0 * T : (t0 + 2) * T],
                    in_=out_pair,
                )

            # ---- idle-time perf probes (results unused) ----
            pv_in = prb.tile([P, T], F16)
            pv_out = prb.tile([P, T], F16)
            nc.vector.memset(pv_in, 1.0)
            nc.vector.tensor_mul(pv_out, p8, pv_in)       # DVE fp8-operand rate
            pg_a = prb.tile([P, T], F16)
            pg_b = prb.tile([P, T], F16)
            pg_o = prb.tile([P, T], F16)
            nc.gpsimd.memset(pg_a, 1.0)
            nc.gpsimd.memset(pg_b, 2.0)
            nc.gpsimd.tensor_mul(pg_o, pg_a, pg_b)        # GPSIMD fp16 rate
    if not nc.is_finalized():
        nc.finalize()
    return nc


def _consts() -> np.ndarray:
    u_excl = np.triu(np.ones((P, P), np.float16), 1)
    u_incl = np.triu(np.ones((P, P), np.float16), 0)
    # E[:, c*C + m] = 1 if m == c else 0  (all rows identical)
    e = np.tile(np.eye(C, dtype=np.float16).reshape(1, C * C), (P, 1))
    return np.ascontiguousarray(np.concatenate([u_excl, u_incl, e], axis=1))


def kernel(rho: np.ndarray, f: np.ndarray) -> np.ndarray:
    global LAST_RESULTS
    if "nc" not in _cached:
        _cached["nc"] = _build_nc()
        _cached["consts"] = _consts()
    nc = _cached["nc"]

    rho16 = np.asarray(rho, dtype=np.float16).reshape(N, S)
    f32 = np.asarray(f, dtype=np.float32).reshape(N, S, C)
    cst = _cached["consts"]
    ntiles = NS // T

    in_maps = []
    for i in range(NCORES):
        sl = slice(i * NS, (i + 1) * NS)
        rho_t = np.ascontiguousarray(
            rho16[sl].reshape(ntiles, T, S).transpose(0, 2, 1)
        ).reshape(ntiles * S, T)
        # [rays, S, C] -> [ntiles, S, C, T] slabs
        f_t = np.ascontiguousarray(
            f32[sl].reshape(ntiles, T, S, C).transpose(0, 2, 3, 1)
        ).reshape(ntiles, S, C * T)
        f16h = f_t[:NH].reshape(NH * S, C * T).astype(np.float16)
        f8 = f_t[NH:].reshape((ntiles - NH) * S, C * T).astype(
            ml_dtypes.float8_e3m4
        )
        in_maps.append(
            {"rho": rho_t, "f16h": f16h, "f8": f8, "consts": cst}
        )
    res = run_bass_kernel_spmd(nc, in_maps, list(range(NCORES)), trace=TRACE)
    LAST_RESULTS = res
    out = np.concatenate(
        [res.results[i]["out"] for i in range(NCORES)], axis=1
    )  # [C, N] fp16
    return out.reshape(C, H, W)[None].astype(np.float32)


# revision 7
# speedup vs baseline: 1.0512x; 1.0512x over previous
"""Trainium2 Bass kernel for nn_DifferentiableProjector (volume rendering).

Math (per ray i, samples s=0..S-1, channels c):
    T_excl[s] = exp(-DT * sum_{s'<s} rho[s'])
    T_incl[s] = exp(-DT * sum_{s'<=s} rho[s'])
    w[s]      = T_excl[s] - T_incl[s]        (= T_excl * alpha)
    out[i,c]  = sum_s w[s] * f[i,s,c]

Sharding: data-parallel over rays, 65536 rays -> 8 cores x 8192 rays.

v3 design (all compute in "transposed space", s on partitions):
  - w for ALL tiles is precomputed up front from rho alone (cumsum via
    triangular matmuls -> fp32 exps -> DVE sub).  This decouples the
    rho->w pipeline from the f pipeline so no cross-tile FIFO convoys
    form (v2 lost ~25 us to one).
  - f tiles 0-1 are fp16 via HWDGE (starts ~2.6us, before the SWDGE
    preamble); tiles 2+ are float8_e3m4 in DRAM upcast to fp16 DURING
    the HBM->SBUF DMA (SWDGE cast path).  SBUF-write side (~400 GB/s)
    is the binding resource; fp8 halves HBM reads and eases it.
  - the big multiply runs on VectorE at 2x fp16
  - segment-reduce over s: accumulating one-hot matmuls on TensorE,
    interleaved per quarter so the reduce trails each quarter-multiply
  - ScalarE evacuates PSUM -> SBUF fp16; out DMAs go on the idle sync
    queue; rho loads in 4 chunks so w-precompute starts early
  - two idle-time probe ops measure GPSIMD fp16 and DVE fp8-operand
    multiply rates for future tuning (no effect on results)
"""

import numpy as np
import ml_dtypes

import concourse.bass as bass
import concourse.tile as tile
from concourse.bacc import Bacc
from concourse import mybir
from concourse.bass_utils import run_bass_kernel_spmd

H, W, S, C = 256, 256, 128, 16
N = H * W
NCORES = 8
NS = N // NCORES          # rays per core
P = 128                   # partitions (= S)
T = 512                   # rays per tile
DT = (6.0 - 2.0) / S
NH = 2                    # head tiles kept fp16 (HWDGE path)

_cached = {}

# test-harness hooks (ignored by grading path)
TRACE = False
LAST_RESULTS = None

F8 = mybir.dt.float8e3
F16 = mybir.dt.float16
F32 = mybir.dt.float32


def _build_nc(ns: int = NS) -> bass.Bass:
    ntiles = ns // T
    nc = Bacc()
    # host supplies tensors pre-transposed AND pre-tiled:
    #   rho  [ntiles*S, T]        fp16
    #   f16h [NH*S, C*T]          fp16      (tiles 0..NH-1)
    #   f8   [(ntiles-NH)*S, C*T] float8e3  (tiles NH..)
    rho_d = nc.dram_tensor("rho", [ntiles * S, T], F16, kind="ExternalInput")
    f16h_d = nc.dram_tensor("f16h", [NH * S, C * T], F16, kind="ExternalInput")
    f8_d = nc.dram_tensor(
        "f8", [(ntiles - NH) * S, C * T], F8, kind="ExternalInput"
    )
    cst_d = nc.dram_tensor("consts", [P, 2 * P + C * C], F16, kind="ExternalInput")
    out_d = nc.dram_tensor("out", [C, ns], F16, kind="ExternalOutput")

    with tile.TileContext(nc) as tc:
        with (
            tc.tile_pool(name="cpool", bufs=1) as cpool,
            tc.tile_pool(name="fpool", bufs=5) as fpool,
            tc.tile_pool(name="tpool", bufs=3) as tpool,
            tc.tile_pool(name="spool", bufs=4) as spool,
            tc.tile_pool(name="opool", bufs=3) as opool,
            tc.tile_pool(name="psc", bufs=2, space="PSUM") as psc,
            tc.tile_pool(name="pso", bufs=2, space="PSUM") as pso,
        ):
            consts = cpool.tile([P, 2 * P + C * C], F16)
            nc.scalar.dma_start(out=consts, in_=cst_d[:, :])
            u_excl = consts[:, 0:P]
            u_incl = consts[:, P : 2 * P]
            e_base = 2 * P

            # rho in 4 chunks so the w-precompute starts after ~0.5 MB
            rho_all = cpool.tile([P, ntiles, T], F16)
            RCH = 4
            for rt in range(0, ntiles, RCH):
                nc.sync.dma_start(
                    out=rho_all[:, rt : rt + RCH, :],
                    in_=rho_d[rt * S : (rt + RCH) * S, :].rearrange(
                        "(t s) i -> s t i", s=S
                    ),
                )

            # f tile DMAs, issued up front (buffer ring gates the tail)
            fts = []
            for t in range(ntiles):
                fT = fpool.tile([P, C, T], F16, tag="fT")
                if t < NH:
                    nc.scalar.dma_start(
                        out=fT,
                        in_=f16h_d[t * S : (t + 1) * S, :].rearrange(
                            "s (c i) -> s c i", c=C
                        ),
                    )
                else:
                    nc.gpsimd.dma_start(
                        out=fT,
                        in_=f8_d[(t - NH) * S : (t - NH + 1) * S, :].rearrange(
                            "s (c i) -> s c i", c=C
                        ),
                    )
                fts.append(fT)

            # ---- w precompute for all tiles (depends only on rho) ----
            w_all = cpool.tile([P, ntiles, T], F16)
            for t in range(ntiles):
                pexc = psc.tile([P, T], F32, tag="pexc")
                pinc = psc.tile([P, T], F32, tag="pinc")
                rhoT = rho_all[:, t, :]
                nc.tensor.matmul(pexc, u_excl, rhoT, start=True, stop=True)
                nc.tensor.matmul(pinc, u_incl, rhoT, start=True, stop=True)
                e1 = spool.tile([P, T], F32, tag="e1")
                e2 = spool.tile([P, T], F32, tag="e2")
                nc.scalar.activation(
                    e1, pexc, mybir.ActivationFunctionType.Exp, scale=-DT
                )
                nc.scalar.activation(
                    e2, pinc, mybir.ActivationFunctionType.Exp, scale=-DT
                )
                nc.vector.tensor_sub(w_all[:, t, :], e1, e2)

            # ---- f pipeline: multiply + per-quarter reduce per pair ----
            NQ = 4
            CQ = C // NQ
            for t0 in range(0, ntiles, 2):
                pair = (t0, t0 + 1)
                tmps = []
                for t in pair:
                    tmp = tpool.tile([P, C, T], F16, tag="tmp")
                    for q in range(NQ):
                        c0 = q * CQ
                        nc.vector.tensor_mul(
                            tmp[:, c0 : c0 + CQ, :],
                            fts[t][:, c0 : c0 + CQ, :],
                            w_all[:, t, None, :].broadcast_to((P, CQ, T)),
                        )
                    tmps.append(tmp)
                psums = [
                    pso.tile([C, T], F32, tag=f"po{i}", name=f"po{i}_{t0}")
                    for i in range(2)
                ]
                for q in range(NQ):
                    for c in range(q * CQ, (q + 1) * CQ):
                        lhs = consts[:, e_base + c * C : e_base + (c + 1) * C]
                        for tmp_t, po in zip(tmps, psums):
                            nc.tensor.matmul(
                                po,
                                lhs,
                                tmp_t[:, c, :],
                                start=(c == 0),
                                stop=(c == C - 1),
                            )
                out_pair = opool.tile(
                    [C, 2 * T], F16, tag="out_pair", name=f"out_pair_{t0}"
                )
                for k, po in enumerate(psums):
                    nc.scalar.activation(
                        out_pair[:, k * T : (k + 1) * T],
                        po,
                        mybir.ActivationFunctionType.Copy,
                    )
                nc.sync.dma_start(
                    out=out_d[:, t0 * T : (t0 + 2) * T],
                    in_=out_pair,
                )
    if not nc.is_finalized():
        nc.finalize()
    return nc


def _consts() -> np.ndarray:
    u_excl = np.triu(np.ones((P, P), np.float16), 1)
    u_incl = np.triu(np.ones((P, P), np.float16), 0)
    # E[:, c*C + m] = 1 if m == c else 0  (all rows identical)
    e = np.tile(np.eye(C, dtype=np.float16).reshape(1, C * C), (P, 1))
    return np.ascontiguousarray(np.concatenate([u_excl, u_incl, e], axis=1))


def kernel(rho: np.ndarray, f: np.ndarray) -> np.ndarray:
    global LAST_RESULTS
    if "nc" not in _cached:
        _cached["nc"] = _build_nc()
        _cached["consts"] = _consts()
    nc = _cached["nc"]

    rho16 = np.asarray(rho, dtype=np.float16).reshape(N, S)
    f32 = np.asarray(f, dtype=np.float32).reshape(N, S, C)
    cst = _cached["consts"]
    ntiles = NS // T

    in_maps = []
    for i in range(NCORES):
        sl = slice(i * NS, (i + 1) * NS)
        rho_t = np.ascontiguousarray(
            rho16[sl].reshape(ntiles, T, S).transpose(0, 2, 1)
        ).reshape(ntiles * S, T)
        # [rays, S, C] -> [ntiles, S, C, T] slabs
        f_t = np.ascontiguousarray(
            f32[sl].reshape(ntiles, T, S, C).transpose(0, 2, 3, 1)
        ).reshape(ntiles, S, C * T)
        f16h = f_t[:NH].reshape(NH * S, C * T).astype(np.float16)
        f8 = f_t[NH:].reshape((ntiles - NH) * S, C * T).astype(
            ml_dtypes.float8_e3m4
        )
        in_maps.append(
            {"rho": rho_t, "f16h": f16h, "f8": f8, "consts": cst}
        )
    res = run_bass_kernel_spmd(nc, in_maps, list(range(NCORES)), trace=TRACE)
    LAST_RESULTS = res
    out = np.concatenate(
        [res.results[i]["out"] for i in range(NCORES)], axis=1
    )  # [C, N] fp16
    return out.reshape(C, H, W)[None].astype(np.float32)


# revision 8
# speedup vs baseline: 1.0571x; 1.0056x over previous
"""Trainium2 Bass kernel for nn_DifferentiableProjector (volume rendering).

Math (per ray i, samples s=0..S-1, channels c):
    T_excl[s] = exp(-DT * sum_{s'<s} rho[s'])
    T_incl[s] = exp(-DT * sum_{s'<=s} rho[s'])
    w[s]      = T_excl[s] - T_incl[s]        (= T_excl * alpha)
    out[i,c]  = sum_s w[s] * f[i,s,c]

Sharding: data-parallel over rays, 65536 rays -> 8 cores x 8192 rays.

v5 design (all compute in "transposed space", s on partitions):
  - f tiles 0-1 are fp16 via HWDGE (starts before the SWDGE preamble);
    tiles 2+ are float8_e3m4 in DRAM upcast to fp16 DURING the HBM->SBUF
    DMA (SWDGE cast path).  The SBUF-write side (~400 GB/s) is the
    binding resource; fp8 halves HBM reads.  All f DMAs are whole-tile
    (16 KB/partition contiguous slabs -> big descriptors) and issued up
    front; the fpool ring provides prefetch depth.
  - rho is host-laid-out [S, ns] so its 4 chunk DMAs are per-partition
    contiguous (no descriptor storm that loses the packet round-robin
    against the f stream).
  - per tile: cumsum over s via triangular matmuls (fp32 PSUM) -> fp32
    exps on ScalarE -> w = e1-e2 on DVE (fp16), emitted immediately
    before that tile's multiply so no engine-FIFO convoys form
  - the big multiply runs on VectorE at 2x fp16
  - segment-reduce over s: accumulating one-hot matmuls on TensorE per
    pair of tiles (shared E_c weight loads), interleaved per quarter so
    the reduce trails each quarter-multiply
  - ScalarE evacuates PSUM -> SBUF fp16; out DMAs go on the sync queue
"""

import numpy as np
import ml_dtypes

import concourse.bass as bass
import concourse.tile as tile
from concourse.bacc import Bacc
from concourse import mybir
from concourse.bass_utils import run_bass_kernel_spmd

H, W, S, C = 256, 256, 128, 16
N = H * W
NCORES = 8
NS = N // NCORES          # rays per core
P = 128                   # partitions (= S)
T = 512                   # rays per tile
DT = (6.0 - 2.0) / S
NH = 2                    # head tiles kept fp16 (HWDGE path)

_cached = {}

# test-harness hooks (ignored by grading path)
TRACE = False
LAST_RESULTS = None

F8 = mybir.dt.float8e3
F16 = mybir.dt.float16
F32 = mybir.dt.float32


def _build_nc(ns: int = NS) -> bass.Bass:
    ntiles = ns // T
    nc = Bacc()
    # host supplies tensors pre-transposed AND pre-tiled:
    #   rho  [S, ns]              fp16      (s-major, fully contiguous)
    #   f16h [NH*S, C*T]          fp16      (tiles 0..NH-1 slabs)
    #   f8   [(ntiles-NH)*S, C*T] float8e3  (tiles NH.. slabs)
    rho_d = nc.dram_tensor("rho", [S, ns], F16, kind="ExternalInput")
    f16h_d = nc.dram_tensor("f16h", [NH * S, C * T], F16, kind="ExternalInput")
    f8_d = nc.dram_tensor(
        "f8", [(ntiles - NH) * S, C * T], F8, kind="ExternalInput"
    )
    cst_d = nc.dram_tensor("consts", [P, 2 * P + C * C], F16, kind="ExternalInput")
    out_d = nc.dram_tensor("out", [C, ns], F16, kind="ExternalOutput")

    with tile.TileContext(nc) as tc:
        with (
            tc.tile_pool(name="cpool", bufs=1) as cpool,
            tc.tile_pool(name="fpool", bufs=6) as fpool,
            tc.tile_pool(name="tpool", bufs=3) as tpool,
            tc.tile_pool(name="spool", bufs=4) as spool,
            tc.tile_pool(name="opool", bufs=3) as opool,
            tc.tile_pool(name="psc", bufs=2, space="PSUM") as psc,
            tc.tile_pool(name="pso", bufs=2, space="PSUM") as pso,
        ):
            consts = cpool.tile([P, 2 * P + C * C], F16)
            nc.scalar.dma_start(out=consts, in_=cst_d[:, :])
            u_excl = consts[:, 0:P]
            u_incl = consts[:, P : 2 * P]
            e_base = 2 * P

            # rho in 4 contiguous chunks (4 KB/partition each)
            rho_all = cpool.tile([P, ns], F16)
            RCH = ns // 4
            for rc in range(4):
                nc.sync.dma_start(
                    out=rho_all[:, rc * RCH : (rc + 1) * RCH],
                    in_=rho_d[:, rc * RCH : (rc + 1) * RCH],
                )

            # f tile DMAs, issued up front (buffer ring gates the tail)
            fts = []
            for t in range(ntiles):
                fT = fpool.tile([P, C, T], F16, tag="fT")
                if t < NH:
                    nc.scalar.dma_start(
                        out=fT,
                        in_=f16h_d[t * S : (t + 1) * S, :].rearrange(
                            "s (c i) -> s c i", c=C
                        ),
                    )
                else:
                    nc.gpsimd.dma_start(
                        out=fT,
                        in_=f8_d[(t - NH) * S : (t - NH + 1) * S, :].rearrange(
                            "s (c i) -> s c i", c=C
                        ),
                    )
                fts.append(fT)

            def tile_front(t):
                """w pipeline + big multiply for tile t."""
                rhoT = rho_all[:, t * T : (t + 1) * T]
                pexc = psc.tile([P, T], F32, tag="pexc")
                pinc = psc.tile([P, T], F32, tag="pinc")
                nc.tensor.matmul(pexc, u_excl, rhoT, start=True, stop=True)
                nc.tensor.matmul(pinc, u_incl, rhoT, start=True, stop=True)

                # exps in fp32 (w = e1 - e2 cancels; fp16 here costs ~4% on w)
                e1 = spool.tile([P, T], F32, tag="e1")
                e2 = spool.tile([P, T], F32, tag="e2")
                nc.scalar.activation(
                    e1, pexc, mybir.ActivationFunctionType.Exp, scale=-DT
                )
                nc.scalar.activation(
                    e2, pinc, mybir.ActivationFunctionType.Exp, scale=-DT
                )
                w = spool.tile([P, T], F16, tag="w")
                nc.vector.tensor_sub(w, e1, e2)

                # tmp[s, c, i] = fT[s, c, i] * w[s, i]
                tmp = tpool.tile([P, C, T], F16, tag="tmp")
                for q in range(4):
                    c0 = q * (C // 4)
                    nc.vector.tensor_mul(
                        tmp[:, c0 : c0 + C // 4, :],
                        fts[t][:, c0 : c0 + C // 4, :],
                        w[:, None, :].broadcast_to((P, C // 4, T)),
                    )
                return tmp

            def tile_back(t0, tmps):
                """Per-quarter reduce + evacuate for a pair of tiles."""
                psums = [
                    pso.tile([C, T], F32, tag=f"po{i}", name=f"po{i}_{t0}")
                    for i in range(2)
                ]
                for q in range(4):
                    for c in range(q * (C // 4), (q + 1) * (C // 4)):
                        lhs = consts[:, e_base + c * C : e_base + (c + 1) * C]
                        for tmp_t, po in zip(tmps, psums):
                            nc.tensor.matmul(
                                po,
                                lhs,
                                tmp_t[:, c, :],
                                start=(c == 0),
                                stop=(c == C - 1),
                            )
                out_pair = opool.tile(
                    [C, 2 * T], F16, tag="out_pair", name=f"out_pair_{t0}"
                )
                for k, po in enumerate(psums):
                    nc.scalar.activation(
                        out_pair[:, k * T : (k + 1) * T],
                        po,
                        mybir.ActivationFunctionType.Copy,
                    )
                nc.sync.dma_start(
                    out=out_d[:, t0 * T : (t0 + 2) * T],
                    in_=out_pair,
                )

            for t0 in range(0, ntiles, 2):
                tmp_a = tile_front(t0)
                tmp_b = tile_front(t0 + 1)
                tile_back(t0, [tmp_a, tmp_b])
    if not nc.is_finalized():
        nc.finalize()
    return nc


def _consts() -> np.ndarray:
    u_excl = np.triu(np.ones((P, P), np.float16), 1)
    u_incl = np.triu(np.ones((P, P), np.float16), 0)
    # E[:, c*C + m] = 1 if m == c else 0  (all rows identical)
    e = np.tile(np.eye(C, dtype=np.float16).reshape(1, C * C), (P, 1))
    return np.ascontiguousarray(np.concatenate([u_excl, u_incl, e], axis=1))


def kernel(rho: np.ndarray, f: np.ndarray) -> np.ndarray:
    global LAST_RESULTS
    if "nc" not in _cached:
        _cached["nc"] = _build_nc()
        _cached["consts"] = _consts()
    nc = _cached["nc"]

    rho16 = np.asarray(rho, dtype=np.float16).reshape(N, S)
    f32 = np.asarray(f, dtype=np.float32).reshape(N, S, C)
    cst = _cached["consts"]
    ntiles = NS // T

    in_maps = []
    for i in range(NCORES):
        sl = slice(i * NS, (i + 1) * NS)
        rho_t = np.ascontiguousarray(rho16[sl].T)            # [S, ns]
        f_t = np.ascontiguousarray(
            f32[sl].reshape(ntiles, T, S, C).transpose(0, 2, 3, 1)
        ).reshape(ntiles, S, C * T)
        f16h = f_t[:NH].reshape(NH * S, C * T).astype(np.float16)
        f8 = f_t[NH:].reshape((ntiles - NH) * S, C * T).astype(
            ml_dtypes.float8_e3m4
        )
        in_maps.append(
            {"rho": rho_t, "f16h": f16h, "f8": f8, "consts": cst}
        )
    res = run_bass_kernel_spmd(nc, in_maps, list(range(NCORES)), trace=TRACE)
    LAST_RESULTS = res
    out = np.concatenate(
        [res.results[i]["out"] for i in range(NCORES)], axis=1
    )  # [C, N] fp16
    return out.reshape(C, H, W)[None].astype(np.float32)


# revision 13
# speedup vs baseline: 1.0622x; 1.0048x over previous
"""Trainium2 Bass kernel for nn_DifferentiableProjector (volume rendering).

Math (per ray i, samples s=0..S-1, channels c):
    T_excl[s] = exp(-DT * sum_{s'<s} rho[s'])
    T_incl[s] = exp(-DT * sum_{s'<=s} rho[s'])
    w[s]      = T_excl[s] - T_incl[s]        (= T_excl * alpha)
    out[i,c]  = sum_s w[s] * f[i,s,c]

Sharding: data-parallel over rays, 65536 rays -> 8 cores x 8192 rays.

v10 = v6 plus whole-tile granularity: each f tile is ONE contiguous\nDMA (16 KB/partition descriptors) and ONE DVE multiply (FD 8192 at\n2x), cutting per-op overhead on both the rings and VectorE.\nv6 notes:
  - out is fp16 (host upcasts); saves write+read ring time
  - rho is host-laid-out [S, ns] so its upfront load is one fully
    contiguous DMA (16 KB/partition rows, big descriptors)
Everything else identical to v1: fp16 f pre-tiled slabs in 4
channel-chunks alternating the two HWDGE queues, cumsum via triangular
matmuls, fp32 exps, DVE 2x multiply, one-hot reduce matmuls with
pair-shared weight loads, ScalarE PSUM evacuation.
"""

import numpy as np

import concourse.bass as bass
import concourse.tile as tile
from concourse.bacc import Bacc
from concourse import mybir
from concourse.bass_utils import run_bass_kernel_spmd

H, W, S, C = 256, 256, 128, 16
N = H * W
NCORES = 8
NS = N // NCORES          # rays per core
P = 128                   # partitions (= S)
T = 512                   # rays per tile
DT = (6.0 - 2.0) / S

_cached = {}

# test-harness hooks (ignored by grading path)
TRACE = False
LAST_RESULTS = None

F16 = mybir.dt.float16
F32 = mybir.dt.float32


def _build_nc(ns: int = NS) -> bass.Bass:
    ntiles = ns // T
    nc = Bacc()
    # host supplies tensors pre-transposed AND pre-tiled:
    #   rho [S, ns]         (s-major, fully contiguous)
    #   f   [ntiles*S, C*T] (tile t rows = f[s, (c, i)] slab, contiguous)
    rho_d = nc.dram_tensor("rho", [S, ns], F16, kind="ExternalInput")
    f_d = nc.dram_tensor("f", [ntiles * S, C * T], F16, kind="ExternalInput")
    cst_d = nc.dram_tensor("consts", [P, 2 * P + C * C], F16, kind="ExternalInput")
    out_d = nc.dram_tensor("out", [C, ns], F16, kind="ExternalOutput")

    with tile.TileContext(nc) as tc:
        with (
            tc.tile_pool(name="cpool", bufs=1) as cpool,
            tc.tile_pool(name="fpool", bufs=6) as fpool,
            tc.tile_pool(name="tpool", bufs=3) as tpool,
            tc.tile_pool(name="spool", bufs=4) as spool,
            tc.tile_pool(name="opool", bufs=3) as opool,
            tc.tile_pool(name="psc", bufs=2, space="PSUM") as psc,
            tc.tile_pool(name="pso", bufs=2, space="PSUM") as pso,
        ):
            consts = cpool.tile([P, 2 * P + C * C], F16)
            nc.scalar.dma_start(out=consts, in_=cst_d[:, :])
            u_excl = consts[:, 0:P]
            u_incl = consts[:, P : 2 * P]
            # E_c = consts[:, 2P + 16c : 2P + 16c + 16]: column m one-hot at c
            e_base = 2 * P

            # all rho upfront: one fully-contiguous DMA (16 KB/partition)
            rho_all = cpool.tile([P, ns], F16)
            nc.sync.dma_start(out=rho_all, in_=rho_d[:, :])

            def tile_front(t):
                """DMA loads + w pipeline + big multiply for tile t."""
                # fT[s, c, i]: contiguous slab DMA in 4 channel-chunks so
                # the multiply overlaps the load at quarter-tile granularity;
                # alternate queues to keep both DGE streams busy
                fT = fpool.tile([P, C, T], F16, tag="fT")
                f_eng = nc.sync if t % 2 == 0 else nc.scalar
                f_eng.dma_start(
                    out=fT,
                    in_=f_d[t * S : (t + 1) * S, :].rearrange(
                        "s (c i) -> s c i", c=C
                    ),
                )
                rhoT = rho_all[:, t * T : (t + 1) * T]

                # cumsum over s (partition axis) via triangular matmuls
                pexc = psc.tile([P, T], F32, tag="pexc")
                pinc = psc.tile([P, T], F32, tag="pinc")
                nc.tensor.matmul(pexc, u_excl, rhoT, start=True, stop=True)
                nc.tensor.matmul(pinc, u_incl, rhoT, start=True, stop=True)

                # exps in fp32 (w = e1 - e2 cancels; fp16 here costs ~4% on w)
                e1 = spool.tile([P, T], F32, tag="e1")
                e2 = spool.tile([P, T], F32, tag="e2")
                nc.scalar.activation(
                    e1, pexc, mybir.ActivationFunctionType.Exp, scale=-DT
                )
                nc.scalar.activation(
                    e2, pinc, mybir.ActivationFunctionType.Exp, scale=-DT
                )
                w = spool.tile([P, T], F16, tag="w")
                nc.vector.tensor_sub(w, e1, e2)

                # tmp[s, c, i] = fT[s, c, i] * w[s, i], per loaded slab
                tmp = tpool.tile([P, C, T], F16, tag="tmp")
                nc.vector.tensor_mul(
                    tmp,
                    fT,
                    w[:, None, :].broadcast_to((P, C, T)),
                )
                return tmp

            def tile_back(t, tmp_pair):
                """Reduce + evacuate for a pair of tiles, sharing each E_c
                weight load across both tiles' matmuls."""
                psums = [pso.tile([C, T], F32, tag=f"po{i}", name=f"po{i}_{t}") for i in range(2)]
                for c in range(C):
                    lhs = consts[:, e_base + c * C : e_base + (c + 1) * C]
                    for tmp_t, po in zip(tmp_pair, psums):
                        nc.tensor.matmul(
                            po,
                            lhs,
                            tmp_t[:, c, :],
                            start=(c == 0),
                            stop=(c == C - 1),
                        )
                out_pair = opool.tile([C, 2 * T], F16, tag="out_pair",
                                      name=f"out_pair_{t}")
                for k, po in enumerate(psums):
                    nc.scalar.activation(
                        out_pair[:, k * T : (k + 1) * T],
                        po,
                        mybir.ActivationFunctionType.Copy,
                    )
                # stream this pair's output out now (overlaps later tiles)
                nc.scalar.dma_start(
                    out=out_d[:, t * T : (t + 2) * T],
                    in_=out_pair,
                )

            for t in range(0, ntiles, 2):
                tmp_a = tile_front(t)
                tmp_b = tile_front(t + 1)
                tile_back(t, [tmp_a, tmp_b])
    if not nc.is_finalized():
        nc.finalize()
    return nc


def _consts() -> np.ndarray:
    u_excl = np.triu(np.ones((P, P), np.float16), 1)
    u_incl = np.triu(np.ones((P, P), np.float16), 0)
    # E[:, c*C + m] = 1 if m == c else 0  (all rows identical)
    e = np.tile(np.eye(C, dtype=np.float16).reshape(1, C * C), (P, 1))
    return np.ascontiguousarray(np.concatenate([u_excl, u_incl, e], axis=1))


def kernel(rho: np.ndarray, f: np.ndarray) -> np.ndarray:
    global LAST_RESULTS
    if "nc" not in _cached:
        _cached["nc"] = _build_nc()
        _cached["consts"] = _consts()
    nc = _cached["nc"]

    rho16 = np.asarray(rho, dtype=np.float16).reshape(N, S)
    f16 = np.asarray(f, dtype=np.float16).reshape(N, S, C)
    cst = _cached["consts"]
    ntiles = NS // T

    in_maps = []
    for i in range(NCORES):
        sl = slice(i * NS, (i + 1) * NS)
        rho_t = np.ascontiguousarray(rho16[sl].T)            # [S, ns]
        f_t = np.ascontiguousarray(
            f16[sl].reshape(ntiles, T, S, C).transpose(0, 2, 3, 1)
        ).reshape(ntiles * S, C * T)
        in_maps.append({"rho": rho_t, "f": f_t, "consts": cst})
    res = run_bass_kernel_spmd(nc, in_maps, list(range(NCORES)), trace=TRACE)
    LAST_RESULTS = res
    out = np.concatenate(
        [res.results[i]["out"] for i in range(NCORES)], axis=1
    )  # [C, N] fp16
    return out.reshape(C, H, W)[None].astype(np.float32)


# revision 14
# speedup vs baseline: 1.2367x; 1.1642x over previous
"""Trainium2 Bass kernel for nn_DifferentiableProjector (volume rendering).

Math (per ray i, samples s=0..S-1, channels c):
    T_excl[s] = exp(-DT * sum_{s'<s} rho[s'])
    T_incl[s] = exp(-DT * sum_{s'<=s} rho[s'])
    w[s]      = T_excl[s] - T_incl[s]        (= T_excl * alpha)
    out[i,c]  = sum_s w[s] * f[i,s,c]

Sharding: data-parallel over rays, 65536 rays -> 8 cores x 8192 rays.

v6 = the proven v1 pipeline with two zero-risk ring-time cuts:
  - out is fp16 (host upcasts); saves write+read ring time
  - rho is host-laid-out [S, ns] so its upfront load is one fully
    contiguous DMA (16 KB/partition rows, big descriptors)
Everything else identical to v1: fp16 f pre-tiled slabs in 4
channel-chunks alternating the two HWDGE queues, cumsum via triangular
matmuls, fp32 exps, DVE 2x multiply, one-hot reduce matmuls with
pair-shared weight loads, ScalarE PSUM evacuation.
"""

import numpy as np

import concourse.bass as bass
import concourse.tile as tile
from concourse.bacc import Bacc
from concourse import mybir
from concourse.bass_utils import run_bass_kernel_spmd

H, W, S, C = 256, 256, 128, 16
N = H * W
NCORES = 8
NS = N // NCORES          # rays per core
P = 128                   # partitions (= S)
T = 512                   # rays per tile
DT = (6.0 - 2.0) / S

_cached = {}

# test-harness hooks (ignored by grading path)
TRACE = False
LAST_RESULTS = None

F16 = mybir.dt.float16
F32 = mybir.dt.float32


def _build_nc(ns: int = NS) -> bass.Bass:
    ntiles = ns // T
    nc = Bacc()
    # host supplies tensors pre-transposed AND pre-tiled:
    #   rho [S, ns]         (s-major, fully contiguous)
    #   f   [ntiles*S, C*T] (tile t rows = f[s, (c, i)] slab, contiguous)
    rho_d = nc.dram_tensor("rho", [S, ns], F16, kind="ExternalInput")
    f_d = nc.dram_tensor("f", [ntiles * S, C * T], F16, kind="ExternalInput")
    cst_d = nc.dram_tensor("consts", [P, 2 * P + C * C], F16, kind="ExternalInput")
    out_d = nc.dram_tensor("out", [C, ns], F16, kind="ExternalOutput")

    with tile.TileContext(nc) as tc:
        with (
            tc.tile_pool(name="cpool", bufs=1) as cpool,
            tc.tile_pool(name="fpool", bufs=6) as fpool,
            tc.tile_pool(name="tpool", bufs=3) as tpool,
            tc.tile_pool(name="spool", bufs=4) as spool,
            tc.tile_pool(name="opool", bufs=3) as opool,
            tc.tile_pool(name="psc", bufs=2, space="PSUM") as psc,
            tc.tile_pool(name="pso", bufs=2, space="PSUM") as pso,
        ):
            consts = cpool.tile([P, 2 * P + C * C], F16)
            nc.scalar.dma_start(out=consts, in_=cst_d[:, :])
            u_excl = consts[:, 0:P]
            u_incl = consts[:, P : 2 * P]
            # E_c = consts[:, 2P + 16c : 2P + 16c + 16]: column m one-hot at c
            e_base = 2 * P

            # all rho upfront: one fully-contiguous DMA (16 KB/partition)
            rho_all = cpool.tile([P, ns], F16)
            nc.sync.dma_start(out=rho_all, in_=rho_d[:, :])

            def tile_front(t):
                """DMA loads + w pipeline + big multiply for tile t."""
                # fT[s, c, i]: contiguous slab DMA in 4 channel-chunks so
                # the multiply overlaps the load at quarter-tile granularity;
                # alternate queues to keep both DGE streams busy
                fT = fpool.tile([P, C, T], F16, tag="fT")
                f_eng = nc.sync if t % 2 == 0 else nc.scalar
                f_slab = f_d[t * S : (t + 1) * S, :].rearrange(
                    "s (c i) -> s c i", c=C
                )
                nsplit = 4
                for q in range(nsplit):
                    c0 = q * (C // nsplit)
                    f_eng.dma_start(
                        out=fT[:, c0 : c0 + C // nsplit, :],
                        in_=f_slab[:, c0 : c0 + C // nsplit, :],
                    )
                rhoT = rho_all[:, t * T : (t + 1) * T]

                # cumsum over s (partition axis) via triangular matmuls
                pexc = psc.tile([P, T], F32, tag="pexc")
                pinc = psc.tile([P, T], F32, tag="pinc")
                nc.tensor.matmul(pexc, u_excl, rhoT, start=True, stop=True)
                nc.tensor.matmul(pinc, u_incl, rhoT, start=True, stop=True)

                # exps in fp32 (w = e1 - e2 cancels; fp16 here costs ~4% on w)
                e1 = spool.tile([P, T], F32, tag="e1")
                e2 = spool.tile([P, T], F32, tag="e2")
                nc.scalar.activation(
                    e1, pexc, mybir.ActivationFunctionType.Exp, scale=-DT
                )
                nc.scalar.activation(
                    e2, pinc, mybir.ActivationFunctionType.Exp, scale=-DT
                )
                w = spool.tile([P, T], F16, tag="w")
                nc.vector.tensor_sub(w, e1, e2)

                # tmp[s, c, i] = fT[s, c, i] * w[s, i], per loaded slab
                tmp = tpool.tile([P, C, T], F16, tag="tmp")
                for q in range(nsplit):
                    c0 = q * (C // nsplit)
                    nc.vector.tensor_mul(
                        tmp[:, c0 : c0 + C // nsplit, :],
                        fT[:, c0 : c0 + C // nsplit, :],
                        w[:, None, :].broadcast_to((P, C // nsplit, T)),
                    )
                return tmp

            def tile_back(t, tmp_pair):
                """Reduce + evacuate for a pair of tiles, sharing each E_c
                weight load across both tiles' matmuls."""
                psums = [pso.tile([C, T], F32, tag=f"po{i}", name=f"po{i}_{t}") for i in range(2)]
                for c in range(C):
                    lhs = consts[:, e_base + c * C : e_base + (c + 1) * C]
                    for tmp_t, po in zip(tmp_pair, psums):
                        nc.tensor.matmul(
                            po,
                            lhs,
                            tmp_t[:, c, :],
                            start=(c == 0),
                            stop=(c == C - 1),
                        )
                out_pair = opool.tile([C, 2 * T], F16, tag="out_pair",
                                      name=f"out_pair_{t}")
                for k, po in enumerate(psums):
                    nc.scalar.activation(
                        out_pair[:, k * T : (k + 1) * T],
                        po,
                        mybir.ActivationFunctionType.Copy,
                    )
                # stream this pair's output out now (overlaps later tiles)
                nc.scalar.dma_start(
                    out=out_d[:, t * T : (t + 2) * T],
                    in_=out_pair,
                )

            for t in range(0, ntiles, 2):
                tmp_a = tile_front(t)
                tmp_b = tile_front(t + 1)
                tile_back(t, [tmp_a, tmp_b])
    if not nc.is_finalized():
        nc.finalize()
    return nc


def _consts() -> np.ndarray:
    u_excl = np.triu(np.ones((P, P), np.float16), 1)
    u_incl = np.triu(np.ones((P, P), np.float16), 0)
    # E[:, c*C + m] = 1 if m == c else 0  (all rows identical)
    e = np.tile(np.eye(C, dtype=np.float16).reshape(1, C * C), (P, 1))
    return np.ascontiguousarray(np.concatenate([u_excl, u_incl, e], axis=1))


def kernel(rho: np.ndarray, f: np.ndarray) -> np.ndarray:
    global LAST_RESULTS
    if "nc" not in _cached:
        _cached["nc"] = _build_nc()
        _cached["consts"] = _consts()
    nc = _cached["nc"]

    rho16 = np.asarray(rho, dtype=np.float16).reshape(N, S)
    f16 = np.asarray(f, dtype=np.float16).reshape(N, S, C)
    cst = _cached["consts"]
    ntiles = NS // T

    in_maps = []
    for i in range(NCORES):
        sl = slice(i * NS, (i + 1) * NS)
        rho_t = np.ascontiguousarray(rho16[sl].T)            # [S, ns]
        f_t = np.ascontiguousarray(
            f16[sl].reshape(ntiles, T, S, C).transpose(0, 2, 3, 1)
        ).reshape(ntiles * S, C * T)
        in_maps.append({"rho": rho_t, "f": f_t, "consts": cst})
    res = run_bass_kernel_spmd(nc, in_maps, list(range(NCORES)), trace=TRACE)
    LAST_RESULTS = res
    out = np.concatenate(
        [res.results[i]["out"] for i in range(NCORES)], axis=1
    )  # [C, N] fp16
    return out.reshape(C, H, W)[None].astype(np.float32)
